# revision 11
# baseline (speedup 1.0000x reference)
"""Trainium2 Bass kernel for GNN message passing (gather + segment_sum).

out[i] = sum_{e: dst[e]==i} x[src[e]]   with x [100000, 64] f32,
edge_index [2, 1600000] int64.

Strategy (8 NeuronCores, SPMD):
  - Destination nodes sharded across cores: core c owns dst rows
    [c*12500, (c+1)*12500), padded to a 12544-row output slab whose row
    order is chosen so every device write is contiguous (host un-permutes).
  - Source nodes are split into 4 chunks of 25000 rows so dma_gather's
    int16 indices stay in range. Each chunk region in HBM also carries a
    zero pad row and per-level scratch rows (see below).
  - Host sorts edges by (dst-core, src-chunk, dst) and assigns each node
    4 "slots" per chunk per level: level 1 holds in-edge ranks 0-3 (or
    0-2 plus a pointer), level L>=2 holds ranks 3(L-1)..3L-1 plus a
    pointer to level L+1. A pointer is the scratch row where the deeper
    level's partial sum is written, so high-degree nodes chain through
    levels and no scatter operation is ever needed.
  - Device: levels run deepest-first; each is a dma_gather (256B rows,
    descriptor generation spread over the 4 SWDGE queues = 4 Q7 core
    pairs), a strided 4-plane vector-engine reduction, and one contiguous
    DMA (scratch rows for levels >= 2, output slab rows for level 1).
"""

import sys

if "/opt/trn_rl_repo" not in sys.path:
    sys.path.insert(0, "/opt/trn_rl_repo")

import numpy as np

N = 100000
D = 64
N_CORES = 8
ROWS_PER_CORE = N // N_CORES            # 12500
NODE_TILE = 1792                        # 14 groups of 128 nodes
GROUPS_PER_TILE = NODE_TILE // 128      # 14
N_TILES = 7
ROWS_PAD = NODE_TILE * N_TILES          # 12544
N_CHUNKS = 4
CHUNK = N // N_CHUNKS                   # 25000
PAD_IDX = CHUNK                         # gather index of the zero row
P_SLOTS = 4
TILE_SLOTS = NODE_TILE * P_SLOTS        # 7168 gather indices per (tile, chunk)

_PROG_CACHE = {}


def _wrap16(a):
    """[..., L] int -> [..., 128, L/16] int16 in the dma_gather index layout:
    position i at [i % 16, i // 16], replicated to all 4 queue core pairs."""
    a = np.ascontiguousarray(a.astype(np.int16))
    L = a.shape[-1]
    assert L % 16 == 0
    t = a.reshape(a.shape[:-1] + (L // 16, 16))
    t = np.swapaxes(t, -1, -2)
    reps = (1,) * (a.ndim - 1) + (8, 1)
    return np.ascontiguousarray(np.tile(t, reps))


def _slab_row(n):
    """Node index within a core -> output slab row (makes tile DMAs contiguous)."""
    t = n // NODE_TILE
    w = n % NODE_TILE
    g = w // 128
    r = w % 128
    return t * NODE_TILE + r * GROUPS_PER_TILE + g


def _gather_order(A):
    """[..., nodes(G*128), 4] slots -> flat gather list order (g, k, r)."""
    G = A.shape[-2] // 128
    A = A.reshape(A.shape[:-2] + (G, 128, P_SLOTS))
    A = np.swapaxes(A, -1, -2)  # (..., G, 4, 128)
    return A.reshape(A.shape[:-3] + (G * 128 * P_SLOTS,))


def _host_prep(x, edge_index):
    src = np.asarray(edge_index[0], dtype=np.int64)
    dst = np.asarray(edge_index[1], dtype=np.int64)
    E = src.shape[0]

    core = dst // ROWS_PER_CORE
    n_loc = dst % ROWS_PER_CORE
    chunk = src // CHUNK
    s_loc = (src % CHUNK).astype(np.int32)

    combo = core * N_CHUNKS + chunk
    gkey = combo * ROWS_PER_CORE + n_loc
    order = np.argsort(gkey, kind="stable")
    gs = gkey[order]
    sl = s_loc[order]

    first = np.empty(E, dtype=bool)
    first[0] = True
    np.not_equal(gs[1:], gs[:-1], out=first[1:])
    gstart = np.flatnonzero(first)
    gid = np.cumsum(first) - 1
    rank = np.arange(E, dtype=np.int64) - gstart[gid]

    deg = np.bincount(gkey, minlength=32 * ROWS_PER_CORE).reshape(32, ROWS_PER_CORE)
    e_combo = gs // ROWS_PER_CORE
    e_node = gs % ROWS_PER_CORE
    e_deg = deg[e_combo, e_node]

    # level of each edge: min(rank//3 + 1, n_levels(deg));
    # n_levels(d) = 1 if d<=4 else 1 + ceil((d-4)/3)
    e_nlvl = np.where(e_deg <= 4, 1, 1 + (np.maximum(e_deg, 5) - 4 + 2) // 3)
    e_lvl = np.minimum(rank // 3 + 1, e_nlvl)
    e_slot = rank - 3 * (e_lvl - 1)

    max_lvl = int(e_lvl.max()) if E else 1

    # level membership/positions, sizes (common across combos), scratch offsets
    lv_pos = [None, None]
    lv_S = [None, None]
    for lv in range(2, max_lvl + 1):
        m = deg > 3 * lv - 2          # [32, 12500]
        cnt = m.sum(axis=1)
        G = int(-(-cnt.max() // 128))
        lv_pos.append(np.cumsum(m, axis=1) - 1)
        lv_S.append(G * 128)

    off = [None, None]
    cur = CHUNK + 1
    for lv in range(2, max_lvl + 1):
        off.append(cur)
        cur += lv_S[lv]
    chunk_region = cur
    assert chunk_region <= 32767, chunk_region

    # ---- slot tables ----
    A = [None, np.full((32, ROWS_PAD, P_SLOTS), PAD_IDX, np.int16)]
    for lv in range(2, max_lvl + 1):
        A.append(np.full((32, lv_S[lv], P_SLOTS), PAD_IDX, np.int16))

    for lv in range(1, max_lvl + 1):
        m = e_lvl == lv
        ec, en, ek, ev = e_combo[m], e_node[m], e_slot[m], sl[m]
        if lv == 1:
            A[1][ec, en, ek] = ev
        else:
            A[lv][ec, lv_pos[lv][ec, en], ek] = ev

    # pointer slots: node at level lv that continues to lv+1 -> slot 3 = scratch
    # row; scratch rows are stored r-major: pos p -> (p % 128) * G + p // 128
    for lv in range(1, max_lvl):
        deeper = deg > 3 * lv + 1
        ci, ni = np.nonzero(deeper)
        p_ = lv_pos[lv + 1][ci, ni]
        G_ = lv_S[lv + 1] // 128
        ptr = off[lv + 1] + (p_ % 128) * G_ + p_ // 128
        if lv == 1:
            A[1][ci, ni, 3] = ptr
        else:
            A[lv][ci, lv_pos[lv][ci, ni], 3] = ptr

    idx1 = _wrap16(_gather_order(A[1])).reshape(8, N_CHUNKS, 128, -1)
    lv_idx = [None, None]
    for lv in range(2, max_lvl + 1):
        lv_idx.append(_wrap16(_gather_order(A[lv])).reshape(8, N_CHUNKS, 128, -1))

    # ---- x_dev with per-chunk scratch regions ----
    x = np.asarray(x, dtype=np.float32)
    x_dev = np.zeros((N_CHUNKS * chunk_region, D), np.float32)
    for c in range(N_CHUNKS):
        x_dev[c * chunk_region : c * chunk_region + CHUNK] = x[c * CHUNK : (c + 1) * CHUNK]

    sizes = tuple(lv_S[2:])
    return x_dev, idx1, lv_idx, sizes, chunk_region


def _build_program(sizes, chunk_region):
    """sizes: scratch rows per level (level 2 first)."""
    import concourse.tile as tile
    from concourse import bacc, mybir

    f32 = mybir.dt.float32
    i16 = mybir.dt.int16
    add = mybir.AluOpType.add

    nc = bacc.Bacc(
        "TRN2",
        target_bir_lowering=False,
        debug=False,
        enable_asserts=False,
        num_devices=N_CORES,
        num_swdge_queues=4,
    )
    x_t = nc.dram_tensor("x_dev", [N_CHUNKS * chunk_region, D], f32, kind="ExternalInput")
    idx1_t = [
        nc.dram_tensor(f"idx1_c{c}", [128, N_TILES * TILE_SLOTS // 16], i16, kind="ExternalInput")
        for c in range(N_CHUNKS)
    ]
    lv_t = []
    for li, S in enumerate(sizes):
        lv_t.append(
            [
                nc.dram_tensor(f"idx_l{li}_c{c}", [128, S * P_SLOTS // 16], i16, kind="ExternalInput")
                for c in range(N_CHUNKS)
            ]
        )
    out_t = nc.dram_tensor("out", [ROWS_PAD, D], f32, kind="ExternalOutput")

    regions = [x_t.ap()[c * chunk_region : (c + 1) * chunk_region] for c in range(N_CHUNKS)]
    out_ap = out_t.ap()

    offs = []
    cur = CHUNK + 1
    for S in sizes:
        offs.append(cur)
        cur += S

    IDX_COLS = TILE_SLOTS // 16
    STAGE_FREE = GROUPS_PER_TILE * P_SLOTS * D

    with tile.TileContext(nc) as tc:
        with (
            tc.tile_pool(name="idxr", bufs=1) as idxr_pool,
            tc.tile_pool(name="stage", bufs=7) as stage_pool,
            tc.tile_pool(name="tmp", bufs=2) as tmp_pool,
            tc.tile_pool(name="part", bufs=2) as part_pool,
            tc.tile_pool(name="lvp", bufs=1) as lvp_pool,
            tc.tile_pool(name="accp", bufs=1) as acc_pool,
        ):
            qn = [0]

            def next_q():
                q = qn[0]
                qn[0] = (q + 1) % N_CHUNKS
                return q

            def reduce4(stg, gsz, dst_view):
                sv = stg[:].rearrange("p (g k f) -> p g k f", k=P_SLOTS, f=D)
                t1 = tmp_pool.tile([128, GROUPS_PER_TILE * D], f32, tag="t1")
                t2 = tmp_pool.tile([128, GROUPS_PER_TILE * D], f32, tag="t2")
                v1 = t1[:, : gsz * D].rearrange("p (g f) -> p g f", f=D)
                v2 = t2[:, : gsz * D].rearrange("p (g f) -> p g f", f=D)
                nc.any.tensor_tensor(v1, sv[:, :, 0, :], sv[:, :, 1, :], op=add)
                nc.any.tensor_tensor(v2, sv[:, :, 2, :], sv[:, :, 3, :], op=add)
                nc.any.tensor_tensor(dst_view, v1, v2, op=add)

            acc = acc_pool.tile([128, N_TILES * GROUPS_PER_TILE * D], f32)

            for c in range(N_CHUNKS):
                idx1_sb = idxr_pool.tile(
                    [128, N_TILES * TILE_SLOTS // 16], i16, tag=f"idx1_{c}"
                )
                nc.sync.dma_start(idx1_sb[:], idx1_t[c].ap()[:])
                lv_sb = []
                for li, S in enumerate(sizes):
                    t_ = idxr_pool.tile([128, S * P_SLOTS // 16], i16, tag=f"lv{li}_{c}")
                    nc.sync.dma_start(t_[:], lv_t[li][c].ap()[:])
                    lv_sb.append(t_)

                # levels for this chunk, deepest first
                for li in range(len(sizes) - 1, -1, -1):
                    S = sizes[li]
                    G = S // 128
                    pr = lvp_pool.tile([128, G * D], f32, tag="lvpart")
                    prv = pr[:].rearrange("p (g f) -> p g f", f=D)
                    for g0 in range(0, G, GROUPS_PER_TILE):
                        g1 = min(G, g0 + GROUPS_PER_TILE)
                        gsz = g1 - g0
                        stg = stage_pool.tile([128, STAGE_FREE], f32, tag="stage")
                        nc.gpsimd.dma_gather(
                            stg[:, : gsz * P_SLOTS * D].rearrange("p (s f) -> p s f", f=D),
                            regions[c],
                            lv_sb[li][:, g0 * 32 : g1 * 32],
                            gsz * 128 * P_SLOTS,
                            gsz * 128 * P_SLOTS,
                            D,
                            single_packet=False,
                            queue_num=next_q(),
                        )
                        reduce4(stg[:, : gsz * P_SLOTS * D], gsz, prv[:, g0:g1, :])
                    dview = regions[c][offs[li] : offs[li] + S].rearrange(
                        "(r g) f -> r (g f)", r=128
                    )
                    nc.sync.dma_start(dview, pr[:])

                # main tiles for this chunk, accumulated into acc
                for t in range(N_TILES):
                    st = stage_pool.tile([128, STAGE_FREE], f32, tag="stage")
                    nc.gpsimd.dma_gather(
                        st[:].rearrange("p (s f) -> p s f", f=D),
                        regions[c],
                        idx1_sb[:, t * IDX_COLS : (t + 1) * IDX_COLS],
                        TILE_SLOTS,
                        TILE_SLOTS,
                        D,
                        single_packet=False,
                        queue_num=next_q(),
                    )
                    a_sl = acc[:, t * GROUPS_PER_TILE * D : (t + 1) * GROUPS_PER_TILE * D]
                    if c == 0:
                        reduce4(st, GROUPS_PER_TILE, a_sl.rearrange("p (g f) -> p g f", f=D))
                    else:
                        pc = part_pool.tile([128, GROUPS_PER_TILE * D], f32, tag="part")
                        reduce4(st, GROUPS_PER_TILE, pc[:].rearrange("p (g f) -> p g f", f=D))
                        nc.any.tensor_tensor(a_sl, a_sl, pc[:], op=add)
                    if c == N_CHUNKS - 1:
                        dview = out_ap[t * NODE_TILE : (t + 1) * NODE_TILE].rearrange(
                            "(r g) f -> r (g f)", r=128
                        )
                        nc.sync.dma_start(dview, a_sl)

    nc.compile()
    return nc


def kernel(x, edge_index):
    from concourse import bass_utils

    x = np.asarray(x, dtype=np.float32)
    edge_index = np.asarray(edge_index)

    x_dev, idx1, lv_idx, sizes, chunk_region = _host_prep(x, edge_index)
    sig = (sizes, chunk_region)
    nc = _PROG_CACHE.get(sig)
    if nc is None:
        nc = _build_program(sizes, chunk_region)
        _PROG_CACHE[sig] = nc

    in_maps = []
    for core in range(N_CORES):
        m = {"x_dev": x_dev}
        for c in range(N_CHUNKS):
            m[f"idx1_c{c}"] = idx1[core, c]
        for li in range(len(sizes)):
            for c in range(N_CHUNKS):
                m[f"idx_l{li}_c{c}"] = lv_idx[li + 2][core, c]
        in_maps.append(m)

    res = bass_utils.run_bass_kernel_spmd(nc, in_maps, core_ids=list(range(N_CORES)))

    perm = _slab_row(np.arange(ROWS_PER_CORE))
    out = np.empty((N, D), np.float32)
    for core in range(N_CORES):
        slab = res.results[core]["out"]
        out[core * ROWS_PER_CORE : (core + 1) * ROWS_PER_CORE] = slab[perm]
    return out


# revision 12
# speedup vs baseline: 1.8760x; 1.8760x over previous
"""Trainium2 Bass kernel for GNN message passing (gather + segment_sum).

out[i] = sum_{e: dst[e]==i} x[src[e]]   with x [100000, 64] f32,
edge_index [2, 1600000] int64.

Strategy (8 NeuronCores, SPMD):
  - Destination nodes sharded across cores: core c owns dst rows
    [c*12500, (c+1)*12500), padded to a 12544-row output slab whose row
    order is chosen so every device write is contiguous (host un-permutes).
  - Source nodes are split into 4 chunks of 25000 rows so dma_gather's
    int16 indices stay in range. Each chunk region in HBM also carries a
    zero pad row and per-level scratch rows (see below).
  - Host sorts edges by (dst-core, src-chunk, dst) and assigns each node
    4 "slots" per chunk per level: level 1 holds in-edge ranks 0-3 (or
    0-2 plus a pointer), level L>=2 holds ranks 3(L-1)..3L-1 plus a
    pointer to level L+1. A pointer is the scratch row where the deeper
    level's partial sum is written, so high-degree nodes chain through
    levels and no scatter operation is ever needed.
  - Device: levels run deepest-first; each is a dma_gather (256B rows,
    descriptor generation spread over the 4 SWDGE queues = 4 Q7 core
    pairs), a strided 4-plane vector-engine reduction, and one contiguous
    DMA (scratch rows for levels >= 2, output slab rows for level 1).
"""

import sys

if "/opt/trn_rl_repo" not in sys.path:
    sys.path.insert(0, "/opt/trn_rl_repo")

import numpy as np

N = 100000
D = 64
N_CORES = 8
ROWS_PER_CORE = N // N_CORES            # 12500
NODE_TILE = 1792                        # 14 groups of 128 nodes
GROUPS_PER_TILE = NODE_TILE // 128      # 14
N_TILES = 7
ROWS_PAD = NODE_TILE * N_TILES          # 12544
N_CHUNKS = 4
CHUNK = N // N_CHUNKS                   # 25000
PAD_IDX = CHUNK                         # gather index of the zero row
P_SLOTS = 4
TILE_SLOTS = NODE_TILE * P_SLOTS        # 7168 gather indices per (tile, chunk)

_PROG_CACHE = {}


def _wrap16(a):
    """[..., L] int -> [..., 128, L/16] int16 in the dma_gather index layout:
    position i at [i % 16, i // 16], replicated to all 4 queue core pairs."""
    a = np.ascontiguousarray(a.astype(np.int16))
    L = a.shape[-1]
    assert L % 16 == 0
    t = a.reshape(a.shape[:-1] + (L // 16, 16))
    t = np.swapaxes(t, -1, -2)
    reps = (1,) * (a.ndim - 1) + (8, 1)
    return np.ascontiguousarray(np.tile(t, reps))


def _slab_row(n):
    """Node index within a core -> output slab row (makes tile DMAs contiguous)."""
    t = n // NODE_TILE
    w = n % NODE_TILE
    g = w // 128
    r = w % 128
    return t * NODE_TILE + r * GROUPS_PER_TILE + g


def _gather_order(A):
    """[..., nodes(G*128), 4] slots -> flat gather list order (g, k, r)."""
    G = A.shape[-2] // 128
    A = A.reshape(A.shape[:-2] + (G, 128, P_SLOTS))
    A = np.swapaxes(A, -1, -2)  # (..., G, 4, 128)
    return A.reshape(A.shape[:-3] + (G * 128 * P_SLOTS,))


def _host_prep(x, edge_index):
    src = np.asarray(edge_index[0], dtype=np.int64)
    dst = np.asarray(edge_index[1], dtype=np.int64)
    E = src.shape[0]

    core = dst // ROWS_PER_CORE
    n_loc = dst % ROWS_PER_CORE
    chunk = src // CHUNK
    s_loc = (src % CHUNK).astype(np.int32)

    combo = core * N_CHUNKS + chunk
    gkey = combo * ROWS_PER_CORE + n_loc
    order = np.argsort(gkey, kind="stable")
    gs = gkey[order]
    sl = s_loc[order]

    first = np.empty(E, dtype=bool)
    first[0] = True
    np.not_equal(gs[1:], gs[:-1], out=first[1:])
    gstart = np.flatnonzero(first)
    gid = np.cumsum(first) - 1
    rank = np.arange(E, dtype=np.int64) - gstart[gid]

    deg = np.bincount(gkey, minlength=32 * ROWS_PER_CORE).reshape(32, ROWS_PER_CORE)
    e_combo = gs // ROWS_PER_CORE
    e_node = gs % ROWS_PER_CORE
    e_deg = deg[e_combo, e_node]

    # level of each edge: min(rank//3 + 1, n_levels(deg));
    # n_levels(d) = 1 if d<=4 else 1 + ceil((d-4)/3)
    e_nlvl = np.where(e_deg <= 4, 1, 1 + (np.maximum(e_deg, 5) - 4 + 2) // 3)
    e_lvl = np.minimum(rank // 3 + 1, e_nlvl)
    e_slot = rank - 3 * (e_lvl - 1)

    max_lvl = int(e_lvl.max()) if E else 1

    # level membership/positions, sizes (common across combos), scratch offsets
    lv_pos = [None, None]
    lv_S = [None, None]
    for lv in range(2, max_lvl + 1):
        m = deg > 3 * lv - 2          # [32, 12500]
        cnt = m.sum(axis=1)
        G = int(-(-cnt.max() // 128))
        lv_pos.append(np.cumsum(m, axis=1) - 1)
        lv_S.append(G * 128)

    off = [None, None]
    cur = CHUNK + 1
    for lv in range(2, max_lvl + 1):
        off.append(cur)
        cur += lv_S[lv]
    chunk_region = cur
    assert chunk_region <= 32767, chunk_region

    # ---- slot tables ----
    A = [None, np.full((32, ROWS_PAD, P_SLOTS), PAD_IDX, np.int16)]
    for lv in range(2, max_lvl + 1):
        A.append(np.full((32, lv_S[lv], P_SLOTS), PAD_IDX, np.int16))

    for lv in range(1, max_lvl + 1):
        m = e_lvl == lv
        ec, en, ek, ev = e_combo[m], e_node[m], e_slot[m], sl[m]
        if lv == 1:
            A[1][ec, en, ek] = ev
        else:
            A[lv][ec, lv_pos[lv][ec, en], ek] = ev

    # pointer slots: node at level lv that continues to lv+1 -> slot 3 = scratch
    # row. Scratch rows are written per sub-block of up to GROUPS_PER_TILE
    # groups, r-major within the block: pos p=(g*128+r) ->
    # g0*128 + r*gsz + (g - g0), with g0 = 14*(g//14).
    for lv in range(1, max_lvl):
        deeper = deg > 3 * lv + 1
        ci, ni = np.nonzero(deeper)
        p_ = lv_pos[lv + 1][ci, ni]
        G_ = lv_S[lv + 1] // 128
        g_ = p_ // 128
        r_ = p_ % 128
        g0_ = (g_ // GROUPS_PER_TILE) * GROUPS_PER_TILE
        gsz_ = np.minimum(G_, g0_ + GROUPS_PER_TILE) - g0_
        ptr = off[lv + 1] + g0_ * 128 + r_ * gsz_ + (g_ - g0_)
        if lv == 1:
            A[1][ci, ni, 3] = ptr
        else:
            A[lv][ci, lv_pos[lv][ci, ni], 3] = ptr

    idx1 = _wrap16(_gather_order(A[1])).reshape(8, N_CHUNKS, 128, -1)
    lv_idx = [None, None]
    for lv in range(2, max_lvl + 1):
        lv_idx.append(_wrap16(_gather_order(A[lv])).reshape(8, N_CHUNKS, 128, -1))

    # ---- x_dev with per-chunk scratch regions ----
    x = np.asarray(x, dtype=np.float32)
    x_dev = np.zeros((N_CHUNKS * chunk_region, D), np.float32)
    for c in range(N_CHUNKS):
        x_dev[c * chunk_region : c * chunk_region + CHUNK] = x[c * CHUNK : (c + 1) * CHUNK]

    sizes = tuple(lv_S[2:])
    return x_dev, idx1, lv_idx, sizes, chunk_region


def _build_program(sizes, chunk_region):
    """sizes: scratch rows per level (level 2 first)."""
    import concourse.tile as tile
    from concourse import bacc, mybir

    f32 = mybir.dt.float32
    i16 = mybir.dt.int16
    add = mybir.AluOpType.add

    nc = bacc.Bacc(
        "TRN2",
        target_bir_lowering=False,
        debug=False,
        enable_asserts=False,
        num_devices=N_CORES,
        num_swdge_queues=4,
    )
    x_t = nc.dram_tensor("x_dev", [N_CHUNKS * chunk_region, D], f32, kind="ExternalInput")
    idx1_t = [
        nc.dram_tensor(f"idx1_c{c}", [128, N_TILES * TILE_SLOTS // 16], i16, kind="ExternalInput")
        for c in range(N_CHUNKS)
    ]
    lv_t = []
    for li, S in enumerate(sizes):
        lv_t.append(
            [
                nc.dram_tensor(f"idx_l{li}_c{c}", [128, S * P_SLOTS // 16], i16, kind="ExternalInput")
                for c in range(N_CHUNKS)
            ]
        )
    out_t = nc.dram_tensor("out", [ROWS_PAD, D], f32, kind="ExternalOutput")

    regions = [x_t.ap()[c * chunk_region : (c + 1) * chunk_region] for c in range(N_CHUNKS)]
    out_ap = out_t.ap()

    offs = []
    cur = CHUNK + 1
    for S in sizes:
        offs.append(cur)
        cur += S

    IDX_COLS = TILE_SLOTS // 16
    STAGE_FREE = GROUPS_PER_TILE * P_SLOTS * D

    with tile.TileContext(nc) as tc:
        with (
            tc.tile_pool(name="idxr", bufs=1) as idxr_pool,
            tc.tile_pool(name="stage", bufs=2) as stage_pool,
            tc.tile_pool(name="tmp", bufs=2) as tmp_pool,
            tc.tile_pool(name="part", bufs=1) as part_pool,
            tc.tile_pool(name="lred", bufs=4) as lred_pool,
            tc.tile_pool(name="outp", bufs=2) as out_pool,
        ):
            def reduce4(stg, gsz, dst_view):
                sv = stg[:].rearrange("p (g k f) -> p g k f", k=P_SLOTS, f=D)
                t1 = tmp_pool.tile([128, GROUPS_PER_TILE * D], f32, tag="t1")
                t2 = tmp_pool.tile([128, GROUPS_PER_TILE * D], f32, tag="t2")
                v1 = t1[:, : gsz * D].rearrange("p (g f) -> p g f", f=D)
                v2 = t2[:, : gsz * D].rearrange("p (g f) -> p g f", f=D)
                nc.any.tensor_tensor(v1, sv[:, :, 0, :], sv[:, :, 1, :], op=add)
                nc.any.tensor_tensor(v2, sv[:, :, 2, :], sv[:, :, 3, :], op=add)
                nc.any.tensor_tensor(dst_view, v1, v2, op=add)

            idx1_sb = []
            for c in range(N_CHUNKS):
                t_ = idxr_pool.tile([128, N_TILES * TILE_SLOTS // 16], i16, tag=f"idx1_{c}")
                nc.sync.dma_start(t_[:], idx1_t[c].ap()[:])
                idx1_sb.append(t_)
            lv_sb = []
            for li, S in enumerate(sizes):
                row = []
                for c in range(N_CHUNKS):
                    t_ = idxr_pool.tile([128, S * P_SLOTS // 16], i16, tag=f"lv{li}_{c}")
                    nc.sync.dma_start(t_[:], lv_t[li][c].ap()[:])
                    row.append(t_)
                lv_sb.append(row)

            # levels, deepest first; every (chunk, sub-block) is independent:
            # gather -> 3 adds -> contiguous write of its own scratch block
            for li in range(len(sizes) - 1, -1, -1):
                S = sizes[li]
                G = S // 128
                for g0 in range(0, G, GROUPS_PER_TILE):
                    g1 = min(G, g0 + GROUPS_PER_TILE)
                    gsz = g1 - g0
                    for c in range(N_CHUNKS):
                        stg = stage_pool.tile([128, gsz * P_SLOTS * D], f32, tag=f"stage{c}")
                        nc.gpsimd.dma_gather(
                            stg[:].rearrange("p (s f) -> p s f", f=D),
                            regions[c],
                            lv_sb[li][c][:, g0 * 32 : g1 * 32],
                            gsz * 128 * P_SLOTS,
                            gsz * 128 * P_SLOTS,
                            D,
                            single_packet=False,
                            queue_num=c,
                        )
                        lr = lred_pool.tile([128, GROUPS_PER_TILE * D], f32, tag="lr")
                        lrv = lr[:, : gsz * D].rearrange("p (g f) -> p g f", f=D)
                        reduce4(stg, gsz, lrv)
                        base = offs[li] + g0 * 128
                        dview = regions[c][base : base + gsz * 128].rearrange(
                            "(r g) f -> r (g f)", r=128
                        )
                        nc.sync.dma_start(dview, lr[:, : gsz * D])

            # level 1: main tiles
            for t in range(N_TILES):
                parts = []
                for c in range(N_CHUNKS):
                    st = stage_pool.tile([128, STAGE_FREE], f32, tag=f"stage{c}")
                    nc.gpsimd.dma_gather(
                        st[:].rearrange("p (s f) -> p s f", f=D),
                        regions[c],
                        idx1_sb[c][:, t * IDX_COLS : (t + 1) * IDX_COLS],
                        TILE_SLOTS,
                        TILE_SLOTS,
                        D,
                        single_packet=False,
                        queue_num=c,
                    )
                    pc = part_pool.tile([128, GROUPS_PER_TILE * D], f32, tag=f"part{c}")
                    reduce4(st, GROUPS_PER_TILE, pc[:].rearrange("p (g f) -> p g f", f=D))
                    parts.append(pc)
                q1 = tmp_pool.tile([128, GROUPS_PER_TILE * D], f32, tag="t1")
                q2 = tmp_pool.tile([128, GROUPS_PER_TILE * D], f32, tag="t2")
                nc.any.tensor_tensor(q1[:], parts[0][:], parts[1][:], op=add)
                nc.any.tensor_tensor(q2[:], parts[2][:], parts[3][:], op=add)
                ot = out_pool.tile([128, GROUPS_PER_TILE * D], f32, tag="out")
                nc.any.tensor_tensor(ot[:], q1[:], q2[:], op=add)
                dview = out_ap[t * NODE_TILE : (t + 1) * NODE_TILE].rearrange(
                    "(r g) f -> r (g f)", r=128
                )
                nc.sync.dma_start(dview, ot[:])

    nc.compile()
    return nc


def kernel(x, edge_index):
    from concourse import bass_utils

    x = np.asarray(x, dtype=np.float32)
    edge_index = np.asarray(edge_index)

    x_dev, idx1, lv_idx, sizes, chunk_region = _host_prep(x, edge_index)
    sig = (sizes, chunk_region)
    nc = _PROG_CACHE.get(sig)
    if nc is None:
        nc = _build_program(sizes, chunk_region)
        _PROG_CACHE[sig] = nc

    in_maps = []
    for core in range(N_CORES):
        m = {"x_dev": x_dev}
        for c in range(N_CHUNKS):
            m[f"idx1_c{c}"] = idx1[core, c]
        for li in range(len(sizes)):
            for c in range(N_CHUNKS):
                m[f"idx_l{li}_c{c}"] = lv_idx[li + 2][core, c]
        in_maps.append(m)

    res = bass_utils.run_bass_kernel_spmd(nc, in_maps, core_ids=list(range(N_CORES)))

    perm = _slab_row(np.arange(ROWS_PER_CORE))
    out = np.empty((N, D), np.float32)
    for core in range(N_CORES):
        slab = res.results[core]["out"]
        out[core * ROWS_PER_CORE : (core + 1) * ROWS_PER_CORE] = slab[perm]
    return out


# revision 13
# speedup vs baseline: 1.9116x; 1.0189x over previous
"""Trainium2 Bass kernel for GNN message passing (gather + segment_sum).

out[i] = sum_{e: dst[e]==i} x[src[e]]   with x [100000, 64] f32,
edge_index [2, 1600000] int64.

Strategy (8 NeuronCores, SPMD):
  - Destination nodes sharded across cores: core c owns dst rows
    [c*12500, (c+1)*12500), padded to a 12544-row output slab whose row
    order is chosen so every device write is contiguous (host un-permutes).
  - Source nodes are split into 4 chunks of 25000 rows so dma_gather's
    int16 indices stay in range. Each chunk region in HBM also carries a
    zero pad row and per-level scratch rows (see below).
  - Host sorts edges by (dst-core, src-chunk, dst) and assigns each node
    4 "slots" per chunk per level: level 1 holds in-edge ranks 0-3 (or
    0-2 plus a pointer), level L>=2 holds ranks 3(L-1)..3L-1 plus a
    pointer to level L+1. A pointer is the scratch row where the deeper
    level's partial sum is written, so high-degree nodes chain through
    levels and no scatter operation is ever needed.
  - Device: levels run deepest-first; each is a dma_gather (256B rows,
    descriptor generation spread over the 4 SWDGE queues = 4 Q7 core
    pairs), a strided 4-plane vector-engine reduction, and one contiguous
    DMA (scratch rows for levels >= 2, output slab rows for level 1).
"""

import sys

if "/opt/trn_rl_repo" not in sys.path:
    sys.path.insert(0, "/opt/trn_rl_repo")

import numpy as np

N = 100000
D = 64
N_CORES = 8
ROWS_PER_CORE = N // N_CORES            # 12500
NODE_TILE = 896                         # 7 groups of 128 nodes
GROUPS_PER_TILE = NODE_TILE // 128      # 7
N_TILES = 14
ROWS_PAD = NODE_TILE * N_TILES          # 12544
N_CHUNKS = 4
CHUNK = N // N_CHUNKS                   # 25000
PAD_IDX = CHUNK                         # gather index of the zero row
P_SLOTS = 4
TILE_SLOTS = NODE_TILE * P_SLOTS        # 7168 gather indices per (tile, chunk)

_PROG_CACHE = {}


def _wrap16(a):
    """[..., L] int -> [..., 128, L/16] int16 in the dma_gather index layout:
    position i at [i % 16, i // 16], replicated to all 4 queue core pairs."""
    a = np.ascontiguousarray(a.astype(np.int16))
    L = a.shape[-1]
    assert L % 16 == 0
    t = a.reshape(a.shape[:-1] + (L // 16, 16))
    t = np.swapaxes(t, -1, -2)
    reps = (1,) * (a.ndim - 1) + (8, 1)
    return np.ascontiguousarray(np.tile(t, reps))


def _slab_row(n):
    """Node index within a core -> output slab row (makes tile DMAs contiguous)."""
    t = n // NODE_TILE
    w = n % NODE_TILE
    g = w // 128
    r = w % 128
    return t * NODE_TILE + r * GROUPS_PER_TILE + g


def _gather_order(A):
    """[..., nodes(G*128), 4] slots -> flat gather list order (g, k, r)."""
    G = A.shape[-2] // 128
    A = A.reshape(A.shape[:-2] + (G, 128, P_SLOTS))
    A = np.swapaxes(A, -1, -2)  # (..., G, 4, 128)
    return A.reshape(A.shape[:-3] + (G * 128 * P_SLOTS,))


def _host_prep(x, edge_index):
    src = np.asarray(edge_index[0], dtype=np.int64)
    dst = np.asarray(edge_index[1], dtype=np.int64)
    E = src.shape[0]

    core = dst // ROWS_PER_CORE
    n_loc = dst % ROWS_PER_CORE
    chunk = src // CHUNK
    s_loc = (src % CHUNK).astype(np.int32)

    combo = core * N_CHUNKS + chunk
    gkey = combo * ROWS_PER_CORE + n_loc
    order = np.argsort(gkey, kind="stable")
    gs = gkey[order]
    sl = s_loc[order]

    first = np.empty(E, dtype=bool)
    first[0] = True
    np.not_equal(gs[1:], gs[:-1], out=first[1:])
    gstart = np.flatnonzero(first)
    gid = np.cumsum(first) - 1
    rank = np.arange(E, dtype=np.int64) - gstart[gid]

    deg = np.bincount(gkey, minlength=32 * ROWS_PER_CORE).reshape(32, ROWS_PER_CORE)
    e_combo = gs // ROWS_PER_CORE
    e_node = gs % ROWS_PER_CORE
    e_deg = deg[e_combo, e_node]

    # level of each edge: min(rank//3 + 1, n_levels(deg));
    # n_levels(d) = 1 if d<=4 else 1 + ceil((d-4)/3)
    e_nlvl = np.where(e_deg <= 4, 1, 1 + (np.maximum(e_deg, 5) - 4 + 2) // 3)
    e_lvl = np.minimum(rank // 3 + 1, e_nlvl)
    e_slot = rank - 3 * (e_lvl - 1)

    max_lvl = int(e_lvl.max()) if E else 1

    # level membership/positions, sizes (common across combos), scratch offsets
    lv_pos = [None, None]
    lv_S = [None, None]
    for lv in range(2, max_lvl + 1):
        m = deg > 3 * lv - 2          # [32, 12500]
        cnt = m.sum(axis=1)
        G = int(-(-cnt.max() // 128))
        lv_pos.append(np.cumsum(m, axis=1) - 1)
        lv_S.append(G * 128)

    off = [None, None]
    cur = CHUNK + 1
    for lv in range(2, max_lvl + 1):
        off.append(cur)
        cur += lv_S[lv]
    chunk_region = cur
    assert chunk_region <= 32767, chunk_region

    # ---- slot tables ----
    A = [None, np.full((32, ROWS_PAD, P_SLOTS), PAD_IDX, np.int16)]
    for lv in range(2, max_lvl + 1):
        A.append(np.full((32, lv_S[lv], P_SLOTS), PAD_IDX, np.int16))

    for lv in range(1, max_lvl + 1):
        m = e_lvl == lv
        ec, en, ek, ev = e_combo[m], e_node[m], e_slot[m], sl[m]
        if lv == 1:
            A[1][ec, en, ek] = ev
        else:
            A[lv][ec, lv_pos[lv][ec, en], ek] = ev

    # pointer slots: node at level lv that continues to lv+1 -> slot 3 = scratch
    # row. Scratch rows are written per sub-block of up to GROUPS_PER_TILE
    # groups, r-major within the block: pos p=(g*128+r) ->
    # g0*128 + r*gsz + (g - g0), with g0 = 14*(g//14).
    for lv in range(1, max_lvl):
        deeper = deg > 3 * lv + 1
        ci, ni = np.nonzero(deeper)
        p_ = lv_pos[lv + 1][ci, ni]
        G_ = lv_S[lv + 1] // 128
        g_ = p_ // 128
        r_ = p_ % 128
        g0_ = (g_ // GROUPS_PER_TILE) * GROUPS_PER_TILE
        gsz_ = np.minimum(G_, g0_ + GROUPS_PER_TILE) - g0_
        ptr = off[lv + 1] + g0_ * 128 + r_ * gsz_ + (g_ - g0_)
        if lv == 1:
            A[1][ci, ni, 3] = ptr
        else:
            A[lv][ci, lv_pos[lv][ci, ni], 3] = ptr

    idx1 = _wrap16(_gather_order(A[1])).reshape(8, N_CHUNKS, 128, -1)
    lv_idx = [None, None]
    for lv in range(2, max_lvl + 1):
        lv_idx.append(_wrap16(_gather_order(A[lv])).reshape(8, N_CHUNKS, 128, -1))

    # ---- x_dev with per-chunk scratch regions ----
    x = np.asarray(x, dtype=np.float32)
    x_dev = np.zeros((N_CHUNKS * chunk_region, D), np.float32)
    for c in range(N_CHUNKS):
        x_dev[c * chunk_region : c * chunk_region + CHUNK] = x[c * CHUNK : (c + 1) * CHUNK]

    sizes = tuple(lv_S[2:])
    return x_dev, idx1, lv_idx, sizes, chunk_region


def _build_program(sizes, chunk_region):
    """sizes: scratch rows per level (level 2 first)."""
    import concourse.tile as tile
    from concourse import bacc, mybir

    f32 = mybir.dt.float32
    i16 = mybir.dt.int16
    add = mybir.AluOpType.add

    nc = bacc.Bacc(
        "TRN2",
        target_bir_lowering=False,
        debug=False,
        enable_asserts=False,
        num_devices=N_CORES,
        num_swdge_queues=4,
    )
    x_t = nc.dram_tensor("x_dev", [N_CHUNKS * chunk_region, D], f32, kind="ExternalInput")
    idx1_t = [
        nc.dram_tensor(f"idx1_c{c}", [128, N_TILES * TILE_SLOTS // 16], i16, kind="ExternalInput")
        for c in range(N_CHUNKS)
    ]
    lv_t = []
    for li, S in enumerate(sizes):
        lv_t.append(
            [
                nc.dram_tensor(f"idx_l{li}_c{c}", [128, S * P_SLOTS // 16], i16, kind="ExternalInput")
                for c in range(N_CHUNKS)
            ]
        )
    out_t = nc.dram_tensor("out", [ROWS_PAD, D], f32, kind="ExternalOutput")

    regions = [x_t.ap()[c * chunk_region : (c + 1) * chunk_region] for c in range(N_CHUNKS)]
    out_ap = out_t.ap()

    offs = []
    cur = CHUNK + 1
    for S in sizes:
        offs.append(cur)
        cur += S

    IDX_COLS = TILE_SLOTS // 16
    STAGE_FREE = GROUPS_PER_TILE * P_SLOTS * D

    with tile.TileContext(nc) as tc:
        with (
            tc.tile_pool(name="idxr", bufs=1) as idxr_pool,
            tc.tile_pool(name="stage", bufs=3) as stage_pool,
            tc.tile_pool(name="tmp", bufs=3) as tmp_pool,
            tc.tile_pool(name="part", bufs=1) as part_pool,
            tc.tile_pool(name="lred", bufs=6) as lred_pool,
            tc.tile_pool(name="outp", bufs=2) as out_pool,
        ):
            def reduce4(stg, gsz, dst_view):
                sv = stg[:].rearrange("p (g k f) -> p g k f", k=P_SLOTS, f=D)
                t1 = tmp_pool.tile([128, GROUPS_PER_TILE * D], f32, tag="t1")
                t2 = tmp_pool.tile([128, GROUPS_PER_TILE * D], f32, tag="t2")
                v1 = t1[:, : gsz * D].rearrange("p (g f) -> p g f", f=D)
                v2 = t2[:, : gsz * D].rearrange("p (g f) -> p g f", f=D)
                nc.any.tensor_tensor(v1, sv[:, :, 0, :], sv[:, :, 1, :], op=add)
                nc.any.tensor_tensor(v2, sv[:, :, 2, :], sv[:, :, 3, :], op=add)
                nc.any.tensor_tensor(dst_view, v1, v2, op=add)

            idx1_sb = []
            for c in range(N_CHUNKS):
                t_ = idxr_pool.tile([128, N_TILES * TILE_SLOTS // 16], i16, tag=f"idx1_{c}")
                nc.sync.dma_start(t_[:], idx1_t[c].ap()[:])
                idx1_sb.append(t_)
            lv_sb = []
            for li, S in enumerate(sizes):
                row = []
                for c in range(N_CHUNKS):
                    t_ = idxr_pool.tile([128, S * P_SLOTS // 16], i16, tag=f"lv{li}_{c}")
                    nc.sync.dma_start(t_[:], lv_t[li][c].ap()[:])
                    row.append(t_)
                lv_sb.append(row)

            # levels, deepest first; every (chunk, sub-block) is independent:
            # gather -> 3 adds -> contiguous write of its own scratch block
            for li in range(len(sizes) - 1, -1, -1):
                S = sizes[li]
                G = S // 128
                for g0 in range(0, G, GROUPS_PER_TILE):
                    g1 = min(G, g0 + GROUPS_PER_TILE)
                    gsz = g1 - g0
                    for c in range(N_CHUNKS):
                        stg = stage_pool.tile([128, gsz * P_SLOTS * D], f32, tag=f"stage{c}")
                        nc.gpsimd.dma_gather(
                            stg[:].rearrange("p (s f) -> p s f", f=D),
                            regions[c],
                            lv_sb[li][c][:, g0 * 32 : g1 * 32],
                            gsz * 128 * P_SLOTS,
                            gsz * 128 * P_SLOTS,
                            D,
                            single_packet=False,
                            queue_num=c,
                        )
                        lr = lred_pool.tile([128, GROUPS_PER_TILE * D], f32, tag="lr")
                        lrv = lr[:, : gsz * D].rearrange("p (g f) -> p g f", f=D)
                        reduce4(stg, gsz, lrv)
                        base = offs[li] + g0 * 128
                        dview = regions[c][base : base + gsz * 128].rearrange(
                            "(r g) f -> r (g f)", r=128
                        )
                        nc.sync.dma_start(dview, lr[:, : gsz * D])

            # level 1: main tiles
            for t in range(N_TILES):
                parts = []
                for c in range(N_CHUNKS):
                    st = stage_pool.tile([128, STAGE_FREE], f32, tag=f"stage{c}")
                    nc.gpsimd.dma_gather(
                        st[:].rearrange("p (s f) -> p s f", f=D),
                        regions[c],
                        idx1_sb[c][:, t * IDX_COLS : (t + 1) * IDX_COLS],
                        TILE_SLOTS,
                        TILE_SLOTS,
                        D,
                        single_packet=False,
                        queue_num=c,
                    )
                    pc = part_pool.tile([128, GROUPS_PER_TILE * D], f32, tag=f"part{c}")
                    reduce4(st, GROUPS_PER_TILE, pc[:].rearrange("p (g f) -> p g f", f=D))
                    parts.append(pc)
                q1 = tmp_pool.tile([128, GROUPS_PER_TILE * D], f32, tag="t1")
                q2 = tmp_pool.tile([128, GROUPS_PER_TILE * D], f32, tag="t2")
                nc.any.tensor_tensor(q1[:], parts[0][:], parts[1][:], op=add)
                nc.any.tensor_tensor(q2[:], parts[2][:], parts[3][:], op=add)
                ot = out_pool.tile([128, GROUPS_PER_TILE * D], f32, tag="out")
                nc.any.tensor_tensor(ot[:], q1[:], q2[:], op=add)
                dview = out_ap[t * NODE_TILE : (t + 1) * NODE_TILE].rearrange(
                    "(r g) f -> r (g f)", r=128
                )
                nc.sync.dma_start(dview, ot[:])

    nc.compile()
    return nc


def kernel(x, edge_index):
    from concourse import bass_utils

    x = np.asarray(x, dtype=np.float32)
    edge_index = np.asarray(edge_index)

    x_dev, idx1, lv_idx, sizes, chunk_region = _host_prep(x, edge_index)
    sig = (sizes, chunk_region)
    nc = _PROG_CACHE.get(sig)
    if nc is None:
        nc = _build_program(sizes, chunk_region)
        _PROG_CACHE[sig] = nc

    in_maps = []
    for core in range(N_CORES):
        m = {"x_dev": x_dev}
        for c in range(N_CHUNKS):
            m[f"idx1_c{c}"] = idx1[core, c]
        for li in range(len(sizes)):
            for c in range(N_CHUNKS):
                m[f"idx_l{li}_c{c}"] = lv_idx[li + 2][core, c]
        in_maps.append(m)

    res = bass_utils.run_bass_kernel_spmd(nc, in_maps, core_ids=list(range(N_CORES)))

    perm = _slab_row(np.arange(ROWS_PER_CORE))
    out = np.empty((N, D), np.float32)
    for core in range(N_CORES):
        slab = res.results[core]["out"]
        out[core * ROWS_PER_CORE : (core + 1) * ROWS_PER_CORE] = slab[perm]
    return out
